# revision 27
# baseline (speedup 1.0000x reference)
"""Trainium2 Bass kernel for nn_NerModel (histogram_binning).

Computes: logits = x @ W + b; p = softmax(logits); global tag histogram;
soft-target CE loss (loss1) + entropy loss (loss2); returns 0.8*l1 + 0.2*l2.

Strategy (8 cores, data-parallel over the 16384 positions):
  - Host: relayout x to k-major (xT [1536, 16384]) so the PE contraction dim
    lands on SBUF partitions with no on-device transposes; shard columns.
  - Device (per core, 2048 positions): logits via PE (x^T chunks stationary,
    W chunks moving), softmax stats on ACT/DVE, per-tag partial sums via
    accumulating stats matmuls against the one-hot tiles:
      row0: s[t]      = sum_pos 1[tag=t]                  (histogram)
      row1: P1raw[t]  = sum_pos 1[tag=t] * sum_t' logit
      row2: P2raw[t]  = sum_pos 1[tag=t] * logit_at_tag
      row3: L2raw[t]  = sum_pos 1[tag=t] * (sum e^l*l)/Z
      row4: P5[t]     = sum_pos 1[tag=t] * logZ_pos       (second pass)
    logZ enters every loss term linearly, so it is factored out and applied
    per-tag on the host; Ln runs once at the end (one ACT table load).
  - Host: sum the 8 [5,50] partials, build the histogram-dependent per-tag
    tables, and reduce ~300 flops to the final scalar.

Math: target = softmax(ys), ys = 1 except val at tag  =>
  sum_t target*logp = (e*sumlogp + (e^val - e)*logp_tag) / (49e + e^val)
with sumlogp = (sum_t l) - T*logZ, logp_tag = l_tag - logZ,
     sum_t p*logp = (sum_t e^l * l)/Z - logZ.
"""

import numpy as np

import concourse.bacc as bacc
import concourse.bass as bass
import concourse.tile as tile
from concourse import mybir
from concourse.bass_utils import run_bass_kernel_spmd

B, S, D2, T = 32, 512, 1536, 50
NCORES = 8
NPOS = B * S            # 16384
MC = NPOS // NCORES     # 2048 positions per core
KC = D2 // 128          # 12 contraction chunks
STW_MAX = 512           # max positions per super-tile
NSUB = STW_MAX // 128   # max subtiles (of 128 positions) per super-tile
NTILE = MC // 128       # 16 position tiles per core
F32 = mybir.dt.float32
BF16 = mybir.dt.bfloat16
USE_BF16 = True         # matmul operands in bf16 (PSUM accum stays f32)
MMDT = BF16 if USE_BF16 else F32
KSPLIT = 3              # xt super-tile DMA split into KSPLIT k-chunk groups
WIDTHS = [512, 512, 512, 256, 256]  # super-tile widths (sum = MC)
assert sum(WIDTHS) == MC
NSUB_MAX = max(WIDTHS) // 128

# const pack layout (free-dim offsets in the single [128, CW] const DMA)
OFF_TAGS = 0            # [128, NTILE] tag value for (tile, partition)
OFF_IOTA = OFF_TAGS + NTILE      # [128, T]   0..T-1 per row
OFF_B = OFF_IOTA + T             # [128, NSUB*T]  b tiled NSUB times
CW = OFF_B + NSUB * T

_CACHE = {}


class _Bacc(bacc.Bacc):
    """Bacc whose activation-table pass only sees full Exp+Ln coverage in
    sets that contain BOTH funcs, so one combined table load serves the
    in-loop Exp stream and the final Ln (instead of a 1.3us mid-kernel
    table swap). Semantically safe: the chosen set genuinely contains
    both functions; we only hide partial-coverage sets from the chooser."""

    def insert_act_table_loads(self):
        import bass_rust as _bass_rust
        from concourse.hw_specs import get_activation_tables

        has_activation = any(
            isinstance(i, mybir.InstActivation)
            for b in self.main_func.blocks
            for i in b.instructions
        )
        if not has_activation:
            return
        AF = mybir.ActivationFunctionType
        tables = list(get_activation_tables(self.m.arch).items())
        filtered = []
        for name, funcs in tables:
            has_exp, has_ln = AF.Exp in funcs, AF.Ln in funcs
            if has_exp != has_ln:
                funcs = funcs - {AF.Exp, AF.Ln}
            filtered.append((name, funcs))
        _bass_rust.insert_act_table_loads(self, filtered)


def _build_nc(reps=1, loop_n=None):
    nc = _Bacc(None, num_devices=NCORES)

    xt = nc.dram_tensor("xt", [D2, MC], MMDT, kind="ExternalInput")
    cpack = nc.dram_tensor("cpack", [128, CW], F32, kind="ExternalInput")
    wmm = nc.dram_tensor("wmm", [128, KC * T], MMDT, kind="ExternalInput")
    partials = nc.dram_tensor("partials", [5, T], F32, kind="ExternalOutput")

    # [128(p = k % 128), KC, MC] view of the k-major x slice
    xt_r = xt[:, :].rearrange("(kc p) m -> p kc m", p=128)

    with tile.TileContext(nc, num_cores=NCORES) as tc:
        if loop_n is not None:
            with tc.For_i(0, loop_n, 1):
                _emit_core(nc, tc, xt_r, cpack, wmm, partials, 0)
        else:
            for rep in range(reps):
                _emit_core(nc, tc, xt_r, cpack, wmm, partials, rep)

    nc.finalize()
    return nc


def _emit_core(nc, tc, xt_r, cpack, wmm, partials, rep):
    AF = mybir.ActivationFunctionType
    ALU = mybir.AluOpType
    if True:
        with (
            tc.tile_pool(name=f"consts{rep}", bufs=1) as consts,
            tc.tile_pool(name=f"xp{rep}", bufs=3) as xp,
            tc.tile_pool(name=f"lps{rep}", bufs=2, space="PSUM") as lps,
            tc.tile_pool(name=f"sps{rep}", bufs=1, space="PSUM") as sps,
            tc.tile_pool(name=f"work{rep}", bufs=2) as work,
            tc.tile_pool(name=f"small{rep}", bufs=2) as small,
        ):
            # W first on the HWDGE queue (PE needs it before anything else);
            # the const pack rides the SWDGE queue in parallel.
            w_sb_t = consts.tile([128, KC * T], MMDT)
            nc.sync.dma_start(out=w_sb_t[:], in_=wmm[:, :])
            w_sb = w_sb_t[:].rearrange("p (kc t) -> p kc t", t=T)
            cp = consts.tile([128, CW], F32)
            nc.gpsimd.dma_start(out=cp[:], in_=cpack[:, :])
            iota_bc = cp[:, OFF_IOTA : OFF_IOTA + T]
            b_bc_full = cp[:, OFF_B : OFF_B + NSUB * T]

            ones4 = consts.tile([128, NSUB], F32)
            nc.vector.memset(ones4[:], 1.0)

            # persistent across the whole kernel
            z_all = consts.tile([128, NTILE], F32)
            o_all = consts.tile([128, NTILE, T], F32)
            stats_ps = sps.tile([4, T], F32)   # rows: ones, lsum, gl, Sel/Z
            p5a_ps = sps.tile([1, T], F32)     # logZ per-tag sums, even tiles
            p5b_ps = sps.tile([1, T], F32)     # logZ per-tag sums, odd tiles

            pending = None  # (is_first, stats, tile0, nsub) — one-iter-late MMs

            m0 = 0
            for st, stw in enumerate(WIDTHS):
                nsub = stw // 128
                if st == 0:
                    ksplits = [2, 4, 6]   # small first piece: PE starts sooner
                elif stw >= 512:
                    ksplits = [4, 4, 4]
                elif stw >= 256:
                    ksplits = [6, 6]
                else:
                    ksplits = [12]
                tile0 = m0 // 128

                xt_sb = xp.tile([128, KC, STW_MAX], MMDT, tag="xt_sb")
                g0 = 0
                for g in ksplits:
                    nc.sync.dma_start(
                        out=xt_sb[:, g0 : g0 + g, 0:stw],
                        in_=xt_r[:, g0 : g0 + g, m0 : m0 + stw],
                    )
                    g0 += g

                l_big = lps.tile([128, NSUB, T], F32, tag="l_big")
                for ms in range(nsub):
                    for kc in range(KC):
                        nc.tensor.matmul(
                            l_big[:, ms, :],
                            lhsT=xt_sb[:, kc, ms * 128 : (ms + 1) * 128],
                            rhs=w_sb[:, kc, :],
                            start=(kc == 0),
                            stop=(kc == KC - 1),
                        )

                lb = work.tile([128, NSUB, T], F32, tag="lb")
                nc.vector.tensor_add(
                    lb[:, 0:nsub, :], l_big[:, 0:nsub, :],
                    b_bc_full[:, 0 : nsub * T].rearrange(
                        "p (j t) -> p j t", t=T
                    ),
                )

                stats = work.tile([128, NSUB * 4], F32, tag="stats")
                st_r = stats[:].rearrange("p (j c) -> p c j", c=4)
                sel4 = small.tile([128, NSUB], F32, tag="sel4")
                recip4 = small.tile([128, NSUB], F32, tag="recip4")

                e_big = work.tile([128, NSUB, T], F32, tag="e_big")
                scr = work.tile([128, NSUB, T], F32, tag="scr")
                scr2 = work.tile([128, NSUB, T], F32, tag="scr2")
                for j in range(nsub):
                    idx = tile0 + j
                    nc.scalar.activation(
                        out=e_big[:, j, :], in_=lb[:, j, :], func=AF.Exp,
                        accum_out=z_all[:, idx : idx + 1],
                    )
                    tcol = cp[:, OFF_TAGS + idx : OFF_TAGS + idx + 1]
                    nc.vector.tensor_scalar(
                        o_all[:, idx, :], iota_bc, tcol, None, ALU.is_equal,
                    )
                    # gl = logit at the tag (accumulated dot with the one-hot)
                    nc.vector.scalar_tensor_tensor(
                        out=scr[:, j, :], in0=iota_bc, scalar=tcol,
                        in1=lb[:, j, :], op0=ALU.is_equal, op1=ALU.mult,
                        accum_out=stats[:, j * 4 + 2 : j * 4 + 3],
                    )
                    # S_el = sum_t e^l * l   (for loss2)
                    nc.vector.scalar_tensor_tensor(
                        out=scr2[:, j, :], in0=e_big[:, j, :], scalar=1.0,
                        in1=lb[:, j, :], op0=ALU.mult, op1=ALU.mult,
                        accum_out=sel4[:, j : j + 1],
                    )

                # lsum straight into stats column 1
                nc.vector.reduce_sum(
                    out=st_r[:, 1, 0:nsub], in_=lb[:, 0:nsub, :],
                    axis=mybir.AxisListType.X,
                )
                nc.vector.reciprocal(
                    out=recip4[:, 0:nsub],
                    in_=z_all[:, tile0 : tile0 + nsub],
                )
                nc.vector.tensor_mul(
                    st_r[:, 3, 0:nsub], sel4[:, 0:nsub], recip4[:, 0:nsub]
                )
                nc.vector.tensor_copy(
                    out=st_r[:, 0, 0:nsub], in_=ones4[:, 0:nsub]
                )

                if pending is not None:
                    _emit_stats(nc, stats_ps, o_all, *pending, is_last=False)
                pending = (st == 0, stats, tile0, nsub)
                m0 += stw

            _emit_stats(nc, stats_ps, o_all, *pending, is_last=True)

            # deferred logZ: one Ln, then per-tag sums of logZ via two
            # parallel accumulation chains of tiny matmuls
            logz_all = consts.tile([128, NTILE], F32)
            nc.scalar.activation(out=logz_all[:], in_=z_all[:], func=AF.Ln)
            half = NTILE // 2
            for i in range(NTILE):
                ps = p5a_ps if i < half else p5b_ps
                nc.tensor.matmul(
                    ps[:, :],
                    lhsT=logz_all[:, i : i + 1],
                    rhs=o_all[:, i, :],
                    start=(i % half == 0),
                    stop=(i % half == half - 1),
                    skip_group_check=True,
                )

            out_sb = consts.tile([4, T], F32)
            out_sb2 = consts.tile([1, T], F32)
            nc.vector.tensor_copy(out=out_sb[:], in_=stats_ps[:])
            nc.vector.tensor_copy(out=out_sb2[:], in_=p5a_ps[:])
            nc.vector.tensor_add(out_sb2[:], out_sb2[:], p5b_ps[:])
            # separate HWDGE queues (SP + ACT) so the two stores overlap
            nc.sync.dma_start(out=partials[0:4, :], in_=out_sb[:])
            nc.scalar.dma_start(out=partials[4:5, :], in_=out_sb2[:])


def _emit_stats(nc, stats_ps, o_all, is_first, stats, tile0, nsub, is_last):
    for j in range(nsub):
        idx = tile0 + j
        nc.tensor.matmul(
            stats_ps[:, :],
            lhsT=stats[:, j * 4 : (j + 1) * 4],
            rhs=o_all[:, idx, :],
            start=(is_first and j == 0),
            stop=(is_last and j == nsub - 1),
            skip_group_check=True,
        )


def _get_nc():
    if "nc" not in _CACHE:
        _CACHE["nc"] = _build_nc()
    return _CACHE["nc"]


def _make_in_maps(x, tags, W, b):
    import ml_dtypes

    mmdt_np = ml_dtypes.bfloat16 if USE_BF16 else np.float32
    x = np.asarray(x, dtype=np.float32).reshape(NPOS, D2)
    tags_f = np.asarray(tags).reshape(NPOS).astype(np.float32)
    W = np.asarray(W, dtype=np.float32)
    b = np.asarray(b, dtype=np.float32)
    xt_full = np.ascontiguousarray(x.T.astype(mmdt_np))  # [D2, NPOS]

    # W k-major: wr[p, kc*T + t] = W[kc*128 + p, t]
    wr = np.ascontiguousarray(
        np.transpose(W.reshape(KC, 128, T), (1, 0, 2))
        .reshape(128, KC * T).astype(mmdt_np)
    )
    iota_bc = np.broadcast_to(np.arange(T, dtype=np.float32), (128, T))
    b_bc = np.broadcast_to(np.tile(b, NSUB), (128, NSUB * T))

    in_maps = []
    for c in range(NCORES):
        xc = np.ascontiguousarray(xt_full[:, c * MC : (c + 1) * MC])
        tg = tags_f[c * MC : (c + 1) * MC].reshape(NTILE, 128).T  # [128, NTILE]
        cpk = np.concatenate([tg, iota_bc, b_bc], axis=1).astype(np.float32)
        cpk = np.ascontiguousarray(cpk)
        assert cpk.shape == (128, CW)
        in_maps.append({"xt": xc, "cpack": cpk, "wmm": wr})
    return in_maps


def _combine(partials_stack, tag_to_score):
    # partials_stack: [NCORES, 5, T] float32
    agg = partials_stack.astype(np.float64).sum(axis=0)
    s, p1raw, p2raw, l2raw, p5 = agg[0], agg[1], agg[2], agg[3], agg[4]
    p1 = p1raw - float(T) * p5          # sum_pos 1[tag=t] * sumlogp_pos
    p2 = p2raw - p5                     # sum_pos 1[tag=t] * logp_tag
    loss2_sum = (l2raw - p5).sum()      # sum_pos (sum_t p*logp)
    score = np.asarray(tag_to_score, dtype=np.float64)
    expo = 1.0 - s / float(S)
    val = score ** expo
    ev = np.exp(val)
    e1 = np.e
    denom = (T - 1) * e1 + ev
    a_tab = e1 / denom
    b_tab = (ev - e1) / denom
    loss1 = -(a_tab @ p1 + b_tab @ p2) / float(NPOS)
    loss2 = -loss2_sum / float(NPOS)
    return np.asarray(0.8 * loss1 + 0.2 * loss2, dtype=np.float32)


def kernel(x, tags, tag_to_score, W, b, _trace=False):
    nc = _get_nc()
    in_maps = _make_in_maps(x, tags, W, b)
    res = run_bass_kernel_spmd(
        nc, in_maps, core_ids=list(range(NCORES)), trace=_trace
    )
    partials = np.stack([r["partials"] for r in res.results])
    out = _combine(partials, tag_to_score)
    if _trace:
        _CACHE["last_results"] = res
    return out
